# revision 29
# baseline (speedup 1.0000x reference)
"""Trainium2 Bass kernel for nn_LowRankLayer_dilation (B=4, C=64, H=W=128).

Math: the reference's rank-3 NMF update collapses exactly (all ranks are
initialized identically), and the eps terms are negligible for this input
distribution, giving:

    h    = relu(W_head @ x)           (per-pixel channel matmul)
    av   = box9(h)                    (3x3 dilation-2 box sum, edge-clamped)
    n'_k = sum_c av_c * h_c(p+d_k)    (9 taps, d in {-2,0,2}^2; n' = 9^2 n)
    q    = sum_k n'_k * h(p+d_k)
    out  = x + (n'_4 / sum_j n'_j^2) * (W_tail @ q)

(All 9/81 scale factors cancel between n'_4/sum n'^2 and q.)

vs the first-pass kernel: the W_tail matmul is folded into the per-k facc
accumulation (no g tensor at all), the f32 residual load is dropped (the
bf16 x already on chip, with its replicate-pad columns baked in on the
host, serves as residual), the output is bf16 (host upcasts), the PE gets
a warm-up/bridge burst so the HAM clock-gate stays open, and the deferred
per-half Cf chain is pumped into the next half's k-loop at points chosen
to avoid engine-FIFO head-of-line blocking.

Sharding: pure data parallel, 8 cores = (batch b, H-half). Each core gets a
68-row halo'd slice packed as 2 channel blocks on 128 partitions:
partition p = c + 64*blk, blk A = slice rows 0..35, blk B = rows 32..67.
Channel reductions/broadcasts run on the PE via block-structured 0/1
matrices. h is stored with 2 replicate-padded columns on each side
(row stride 132), so every dilated tap is a pure strided AP view.
"""
import sys
import contextlib
import numpy as np

sys.path.insert(0, '/opt/trn_rl_repo')

import concourse.bass as bass  # noqa: E402,F401
import concourse.bacc as bacc  # noqa: E402
import concourse.tile as tile  # noqa: E402
import concourse.mybir as mybir  # noqa: E402
from concourse.bass_utils import run_bass_kernel_spmd  # noqa: E402

F32 = mybir.dt.float32
BF16 = mybir.dt.bfloat16
AT = mybir.ActivationFunctionType
OP = mybir.AluOpType

N_CORES = 8
RIN = 36          # per-block input rows (with +-2 halo)
ROUT = 32         # per-block output rows
W = 128
WP = W + 4        # padded row stride for h
FIN = RIN * W     # 4608
FINP = RIN * WP   # 4752 (x with replicated edge columns baked in)
FOUT = ROUT * W   # 4096
HF = 2048         # half of FOUT
OFFS = [(di, dj) for di in (-2, 0, 2) for dj in (-2, 0, 2)]




def _build():
    nc = bacc.Bacc("TRN2", target_bir_lowering=False, debug=False,
                   num_devices=N_CORES)
    xb_ext = nc.dram_tensor("xb", [128, FINP], BF16, kind="ExternalInput").ap()
    w2_ext = nc.dram_tensor("w2", [128, 128], BF16, kind="ExternalInput").ap()
    w3_ext = nc.dram_tensor("w3", [128, 128], BF16, kind="ExternalInput").ap()
    bo_ext = nc.dram_tensor("bo", [128, 128], BF16, kind="ExternalInput").ap()
    sb_ext = nc.dram_tensor("sb", [18, 2], BF16, kind="ExternalInput").ap()
    bc2_ext = nc.dram_tensor("bc2", [34, 128], BF16, kind="ExternalInput").ap()
    y_ext = nc.dram_tensor("y", [128, FOUT], BF16, kind="ExternalOutput").ap()

    with tile.TileContext(nc) as tc, contextlib.ExitStack() as ctx:
        cpool = ctx.enter_context(tc.tile_pool(name="consts", bufs=1))
        big = ctx.enter_context(tc.tile_pool(name="big", bufs=1))
        ppool = ctx.enter_context(tc.tile_pool(name="prod", bufs=7))
        npool = ctx.enter_context(tc.tile_pool(name="nbuf", bufs=4))

        w2 = cpool.tile([128, 128], BF16)
        nc.sync.dma_start(w2[:], w2_ext[:])
        xbt = big.tile([128, FINP], BF16)
        nc.sync.dma_start(xbt[:, 0:512], xb_ext[:, 0:512])
        nc.sync.dma_start(xbt[:, 512:1536], xb_ext[:, 512:1536])
        nc.gpsimd.dma_start(xbt[:, 1536:2560], xb_ext[:, 1536:2560])
        nc.sync.dma_start(xbt[:, 2560:3584], xb_ext[:, 2560:3584])
        nc.gpsimd.dma_start(xbt[:, 3584:4752], xb_ext[:, 3584:4752])
        w3 = cpool.tile([128, 128], BF16)
        nc.sync.dma_start(w3[:], w3_ext[:])
        bo = cpool.tile([128, 128], BF16)
        nc.gpsimd.dma_start(bo[:], bo_ext[:])
        sbm = cpool.tile([18, 2], BF16)
        nc.gpsimd.dma_start(sbm[:], sb_ext[:])
        bc2 = cpool.tile([34, 128], BF16)
        nc.gpsimd.dma_start(bc2[:], bc2_ext[:])

        # h: (RIN, WP) row layout; data at cols 2..129, replicate pads at
        # cols 0,1,130,131. A (di,dj) tap over rows is then a pure strided
        # 3D view with the edge clamp built in.
        hf = big.tile([128, RIN * WP], BF16)
        h3 = hf.rearrange("p (r w) -> p r w", w=WP)
        xb3 = xbt.rearrange("p (r w) -> p r w", w=WP)

        def tap(t3, di, dj, rows=ROUT, r0=2):
            rr = r0 + di
            return t3[:, rr:rr + rows, 2 + dj:2 + dj + W]

        T = big.tile([128, FIN], BF16)
        T3 = T.rearrange("p (r w) -> p r w", w=W)
        av = big.tile([128, FOUT], BF16)
        av3 = av.rearrange("p (r w) -> p r w", w=W)

        # ---- PE warm-up: ~5us of dummy matmuls during the input DMAs so
        # the HAM clock-gate opens (1.2 -> 2.4 GHz) before real work ----
        wsc = cpool.tile([128, 640], BF16)
        nc.vector.memset(wsc[:], 0.0)
        with tc.tile_pool(name="pswarm", bufs=1, space="PSUM") as pswarm:
            wps = pswarm.tile([128, 512], F32)
            for _ in range(14):
                nc.tensor.matmul(wps[:], wsc[:, 0:128], wsc[:, 128:640],
                                 start=True, stop=True)

        # ---- head matmul h = relu(W_head @ x). x carries its replicate
        # pads, so h's pads fall out of the matmul directly and the relu
        # chunks are flat copies. Column-sum T rows chase relu coverage. ----
        def emit_T(r0, r1):
            nr = r1 - r0
            nc.vector.tensor_add(T3[:, r0:r1, :],
                                 tap(h3, -2, -2, nr, 2 + r0),
                                 tap(h3, -2, 0, nr, 2 + r0))
            nc.vector.tensor_add(T3[:, r0:r1, :], T3[:, r0:r1, :],
                                 tap(h3, -2, 2, nr, 2 + r0))

        with tc.tile_pool(name="psmm", bufs=2, space="PSUM") as psmm:
            t_done = 0
            chunks = [(0, 1024), (1024, 2048), (2048, 3072), (3072, 4096),
                      (4096, 4752)]
            for j, (c0, c1) in enumerate(chunks):
                ps = psmm.tile([128, 1024], F32)
                q0 = 0
                while c0 + q0 < c1:
                    w_ = min(512, c1 - c0 - q0)
                    nc.tensor.matmul(ps[:, q0:q0 + w_], w2[:],
                                     xbt[:, c0 + q0:c0 + q0 + w_],
                                     start=True, stop=True)
                    q0 += w_
                nc.scalar.activation(hf[:, c0:c1], ps[:, 0:c1 - c0],
                                     AT.Relu)
                avail = min(RIN, c1 // WP)
                if avail > t_done:
                    emit_T(t_done, avail)
                    t_done = avail
                if j == 2:
                    nc.vector.tensor_add(av[:, 0:HF], T[:, 0:HF],
                                         T[:, 2 * W:2 * W + HF])
                    nc.vector.tensor_add(av[:, 0:HF], av[:, 0:HF],
                                         T[:, 4 * W:4 * W + HF])
            # PE bridge: dummy matmuls so the PE stays busy (and warm)
            # through the DVE-bound box phase
            wps2 = psmm.tile([128, 1024], F32)
            for _ in range(42):
                nc.tensor.matmul(wps2[:, 0:512], wsc[:, 0:128],
                                 wsc[:, 128:640], start=True, stop=True)

        # ---- per-k: n'_k (PE reduce+broadcast), facc = sum_k W_tail@(n'_k
        # h_tap) accumulated on the PE. Two half-passes (16 out-rows each).
        # The Cf / output chain of each half is emitted lagged, inside the
        # next half's k-loop. ----
        nst = cpool.tile([34, FOUT], BF16)      # n' rows, row pair by kr
        # rows 32,33 duplicate the center rows (0,1) so the ch=1 Cf chain
        # stays partition-aligned with its s2/rcp rows at base 32
        nsq = cpool.tile([18, FOUT], BF16)
        facc = big.tile([128, FOUT], BF16)
        cfr = cpool.tile([34, FOUT], BF16)

        with tc.tile_pool(name="psnk", bufs=2, space="PSUM") as psnk, \
                tc.tile_pool(name="psfa", bufs=1, space="PSUM") as psfa, \
                tc.tile_pool(name="rows", bufs=1) as rows:

            def cf_steps(half):
                """Deferred tail for one half: nsq -> s2 (packed [4,1024])
                -> one rcp -> cfr -> cfb broadcast -> residual -> DMA."""
                for ch in range(2):
                    cs = slice(half * HF + ch * 1024,
                               half * HF + (ch + 1) * 1024)
                    if half == 1:
                        nc.vector.tensor_mul(nsq[:, cs], nst[0:18, cs],
                                             nst[0:18, cs])
                    else:
                        nc.scalar.activation(nsq[:, cs], nst[0:18, cs],
                                             AT.Square)
                    nc.scalar.copy(facc[:, cs],
                                   facc_ps_of[half][:, ch * 1024:
                                                     (ch + 1) * 1024])
                yield
                rcp = rows.tile([2, HF], F32, tag="rcp")
                for ch in range(2):
                    sl = slice(half * HF + ch * 1024,
                               half * HF + (ch + 1) * 1024)
                    rs = slice(ch * 1024, (ch + 1) * 1024)
                    s2c = psnk.tile([128, 1024], F32, tag="nk")
                    for q in range(2):
                        c0 = half * HF + ch * 1024 + q * 512
                        nc.tensor.matmul(
                            s2c[0:2, q * 512:(q + 1) * 512], sbm[:],
                            nsq[:, c0:c0 + 512], start=True, stop=True)
                    nc.vector.reciprocal_approx_fast(rcp[:, rs], s2c[0:2, :])
                    nc.vector.tensor_mul(cfr[0:2, sl], nst[0:2, sl],
                                         rcp[:, rs])
                    yield
                for ch in range(2):
                    sl = slice(half * HF + ch * 1024,
                               half * HF + (ch + 1) * 1024)
                    cfb = psnk.tile([128, 1024], F32, tag="nk")
                    for q in range(2):
                        c0 = half * HF + ch * 1024 + q * 512
                        nc.tensor.matmul(cfb[:, q * 512:(q + 1) * 512],
                                         bc2[0:2, :],
                                         cfr[0:2, c0:c0 + 512],
                                         start=True, stop=True)
                    resm = npool.tile([128, 1024], BF16, tag="res")
                    nc.vector.tensor_mul(resm[:], facc[:, sl], cfb[:])
                    ysb = npool.tile([128, 1024], BF16, tag="ysb")
                    r0 = 2 + half * 16 + ch * 8
                    xres = xb3[:, r0:r0 + 8, 2:2 + W]
                    nc.vector.tensor_add(
                        ysb[:].rearrange("p (r w) -> p r w", w=W),
                        resm[:].rearrange("p (r w) -> p r w", w=W), xres)
                    nc.gpsimd.dma_start(y_ext[:, sl], ysb[:])
                    yield

            pending = None                    # deferred cf-chain generator
            facc_ps_of = {}
            prods = {}
            for half in range(2):
                rh = half * 16

                def emit_prod(hh, k):
                    di, dj = OFFS[k]
                    rhh = hh * 16
                    prod = ppool.tile([128, HF], BF16, tag="pp")
                    p3 = prod.rearrange("p (r w) -> p r w", w=W)
                    nc.vector.tensor_mul(
                        p3[:], av3[:, rhh:rhh + 16, :],
                        tap(h3, di, dj, rows=16, r0=2 + rhh))
                    return prod

                if (half, 0) not in prods:
                    prods[(half, 0)] = emit_prod(half, 0)
                facc_ps = psfa.tile([128, HF], F32, tag="facc_ps")
                facc_ps_of[half] = facc_ps
                pks = {}

                def emit_facc(k):
                    pk = pks.pop(k)
                    for q in range(4):
                        c0 = q * 512
                        nc.tensor.matmul(facc_ps[:, c0:c0 + 512], w3[:],
                                         pk[:, c0:c0 + 512],
                                         start=(k == 0), stop=(k == 8))

                for k, (di, dj) in enumerate(OFFS):
                    prod = prods.pop((half, k))
                    nb = npool.tile([128, HF], BF16, tag="nb")
                    for q in range(2):
                        pst = psnk.tile([128, 1024], F32, tag="nk")
                        for u in range(2):
                            c0 = q * 1024 + u * 512
                            nc.tensor.matmul(pst[:, u * 512:(u + 1) * 512],
                                             bo[:], prod[:, c0:c0 + 512],
                                             start=True, stop=True)
                        nc.scalar.copy(nb[:, q * 1024:(q + 1) * 1024], pst[:])
                    kr = (k - 4) % 9          # put k=4 (center) at rows 0..1
                    hs = slice(half * HF, (half + 1) * HF)
                    if k == 8:
                        for ch in range(2):
                            cs = slice(half * HF + ch * 1024,
                                       half * HF + (ch + 1) * 1024)
                            nc.sync.dma_start(
                                nst[2 * kr:2 * kr + 2, cs],
                                nb[0:128:64, ch * 1024:(ch + 1) * 1024])
                    else:
                        nc.sync.dma_start(nst[2 * kr:2 * kr + 2, hs],
                                          nb[0:128:64, :])

                    if half == 0 and k == 1:
                        nc.vector.tensor_add(av[:, HF:FOUT],
                                             T[:, HF:HF + HF],
                                             T[:, HF + 2 * W:HF + 2 * W + HF])
                        nc.vector.tensor_add(av[:, HF:FOUT], av[:, HF:FOUT],
                                             T[:, HF + 4 * W:HF + 4 * W + HF])
                    if k + 1 < 9 and (half, k + 1) not in prods:
                        prods[(half, k + 1)] = emit_prod(half, k + 1)
                    if half == 0 and k >= 7:
                        prods[(1, k - 7)] = emit_prod(1, k - 7)

                    nb3 = nb.rearrange("p (r w) -> p r w", w=W)
                    pk = ppool.tile([128, HF], BF16, tag="pp")
                    p3 = pk.rearrange("p (r w) -> p r w", w=W)
                    nc.vector.tensor_mul(p3[:, 0:8, :], nb3[:, 0:8, :],
                                         tap(h3, di, dj, rows=8, r0=2 + rh))
                    nc.vector.tensor_mul(p3[:, 8:16, :], nb3[:, 8:16, :],
                                         tap(h3, di, dj, rows=8,
                                             r0=2 + rh + 8))
                    pks[k] = pk
                    if k >= 1:
                        emit_facc(k - 1)
                    if pending is not None and k in (1, 2, 3, 5, 7):
                        next(pending, None)
                emit_facc(8)
                if pending is not None:
                    for _ in pending:
                        pass
                pending = cf_steps(half)
            for _ in pending:
                pass

    nc.compile()
    return nc


_NC_CACHE = [None]


def _get_nc():
    if _NC_CACHE[0] is None:
        _NC_CACHE[0] = _build()
    return _NC_CACHE[0]


def _host_prep(x):
    import ml_dtypes
    B, Cc, H, Ww = x.shape
    in_maps = []
    for core in range(N_CORES):
        b, half = core // 2, core % 2
        r0 = 64 * half
        gidx = np.clip(np.arange(r0 - 2, r0 + 66), 0, H - 1)
        xs = x[b][:, gidx, :]                     # (64, 68, 128)
        packed = np.ascontiguousarray(
            np.concatenate([xs[:, 0:36], xs[:, 32:68]], axis=0))
        padded = np.pad(packed, ((0, 0), (0, 0), (2, 2)), mode='edge')
        in_maps.append({
            "xb": padded.reshape(128, FINP).astype(ml_dtypes.bfloat16),
        })
    return in_maps


def _const_maps(W_head, W_tail):
    import ml_dtypes

    def to_bf(a):
        return a.astype(ml_dtypes.bfloat16)

    w2 = np.zeros((128, 128), np.float32)
    w2[:64, :64] = W_head.T
    w2[64:, 64:] = W_head.T
    w3 = np.zeros((128, 128), np.float32)
    w3[:64, :64] = W_tail.T
    w3[64:, 64:] = W_tail.T
    bo = np.zeros((128, 128), np.float32)
    bo[:64, :64] = 1.0
    bo[64:, 64:] = 1.0
    sb = np.zeros((18, 2), np.float32)
    sb[0::2, 0] = 1.0
    sb[1::2, 1] = 1.0
    bc2 = np.zeros((34, 128), np.float32)
    bc2[0, :64] = 1.0
    bc2[1, 64:] = 1.0
    bc2[32, :64] = 1.0
    bc2[33, 64:] = 1.0
    return {"w2": to_bf(w2), "w3": to_bf(w3), "bo": to_bf(bo),
            "sb": to_bf(sb), "bc2": to_bf(bc2)}


def kernel(x, W_head, W_tail):
    x = np.asarray(x, np.float32)
    W_head = np.asarray(W_head, np.float32)
    W_tail = np.asarray(W_tail, np.float32)
    nc = _get_nc()
    consts = _const_maps(W_head, W_tail)
    in_maps = [{**m, **consts} for m in _host_prep(x)]
    res = run_bass_kernel_spmd(nc, in_maps, list(range(N_CORES)))
    out = np.empty_like(x)
    for core in range(N_CORES):
        b, half = core // 2, core % 2
        r0 = 64 * half
        y = res.results[core]["y"].astype(np.float32).reshape(128, ROUT, W)
        out[b, :, r0:r0 + 32, :] = y[:64]
        out[b, :, r0 + 32:r0 + 64, :] = y[64:]
    return out
